# revision 1
# baseline (speedup 1.0000x reference)
"""GCN (3-layer + readout) on 8 Trainium2 NeuronCores.

Strategy (dst-node sharding, 1D graph parallel):
  - Nodes are sharded across 8 cores (6250/core, padded to 6272 = 49 blocks
    of 128).  Each core aggregates messages for the edges whose dst lands in
    its shard.
  - Per layer: transform z = h @ W (PE), scale rows by dinv = deg^-1/2 so
    table rows are dinv[src]*z[src].  Layer 1's table is computed fully
    locally by every core (x is replicated; each core gets a node
    permutation with its own shard first so the self-term slice is uniform
    across the SPMD program).  Layers 2/3 AllGather the shard tables.
  - Edge gathers: one indirect DMA (standard InstDMACopy dynamic-offset
    SWDGE path, int32 row ids, one row per partition) per 128-edge chunk.
  - Scatter-add on the TensorEngine: per chunk a one-hot
    onehot[e, d] = (dst_local[e] == d) is built with one DVE
    tensor_scalar(is_equal) against an iota row; psum[128d, 64] +=
    onehot^T @ msg accumulates the dst-block aggregation.
  - Self loops (PyG GCNConv implicit): the own-shard table slice is kept in
    SBUF and added to the block aggregate before the dst-side dinv scale,
    yielding exactly dinv^2 * z.
  - Host-side preprocessing is strictly index/metadata work (edge bucketing,
    padding, degree counting, node permutation); all float math runs on
    device.
"""

import numpy as np

from concourse import bacc, bass, mybir, tile
from concourse.bass_utils import run_bass_kernel_spmd

# ---------------------------------------------------------------- constants
P = 8                      # cores
N = 50000                  # nodes
IN_DIM = 128
HID = 64
OUT_DIM = 10
BLK = 128

F32 = mybir.dt.float32
I32 = mybir.dt.int32


def _derived():
    shard = N // P
    nblk = (shard + BLK - 1) // BLK
    pads = nblk * BLK
    tbl = P * pads
    return shard, nblk, pads, tbl


def _refresh_dims():
    global SHARD, NBLK, PADS, TBL
    SHARD, NBLK, PADS, TBL = _derived()


SHARD, NBLK, PADS, TBL = _derived()


# ------------------------------------------------------------- host prep
def _preprocess(x, edge_index):
    """Bucket edges into per-(core, dst-block) 128-edge chunks.

    Nodes are bin-packed into the P*NBLK (core, block) bins by in-degree
    (capacity-constrained LPT) so every bin carries ~the same edge count —
    this minimizes the uniform per-block chunk counts, which set the Q7
    gather-instruction floor.
    """
    import heapq

    x = np.asarray(x, np.float32)
    ei = np.asarray(edge_index, np.int64)
    src, dst = ei[0], ei[1]

    degE = np.bincount(dst, minlength=N).astype(np.int64)
    deg = (degE + 1).astype(np.float32)

    NBINS = P * NBLK
    order_n = np.argsort(-degE, kind="stable")
    heap = [(0, b) for b in range(NBINS)]
    heapq.heapify(heap)
    fill = np.zeros(NBINS, np.int64)
    node_bin = np.empty(N, np.int64)
    node_slot = np.empty(N, np.int64)
    for n in order_n:
        while True:
            s, b = heapq.heappop(heap)
            if fill[b] < BLK:
                break
        node_bin[n] = b
        node_slot[n] = fill[b]
        fill[b] += 1
        heapq.heappush(heap, (s + int(degE[n]), b))

    newid = node_bin * BLK + node_slot          # padded global row of each node

    owner = node_bin[dst] // NBLK
    blk = node_bin[dst] % NBLK
    dstl = node_slot[dst].astype(np.float32)
    s_own = node_bin[src] // NBLK
    s_loc = (node_bin[src] % NBLK) * BLK + node_slot[src]   # padded local row

    # chunk slots
    gid = owner * NBLK + blk
    order = np.argsort(gid, kind="stable")
    gid_s = gid[order]
    counts = np.bincount(gid_s, minlength=P * NBLK)
    starts = np.concatenate([[0], np.cumsum(counts)[:-1]])
    pos = np.arange(gid_s.size) - starts[gid_s]

    # per-block chunk count: max over cores (program must be core-uniform)
    C_arr = np.maximum(
        np.ceil(counts.reshape(P, NBLK).max(axis=0) / BLK).astype(np.int64), 1)
    base = np.concatenate([[0], np.cumsum(C_arr)[:-1]])
    T = int(C_arr.sum())

    own_s = gid_s // NBLK
    blk_s = gid_s % NBLK
    slot = base[blk_s] * BLK + pos            # slot within the core's stream

    # AG-table row (rank-ordered layout, layers 2/3)
    row23 = (s_own * PADS + s_loc)[order]
    dstl_s = dstl[order]

    g23 = np.zeros((P, T * BLK), np.int32)
    dv = np.full((P, T * BLK), -1.0, np.float32)
    flat = own_s * (T * BLK) + slot
    g23.reshape(-1)[flat] = row23.astype(np.int32)
    dv.reshape(-1)[flat] = dstl_s

    # layer-1 table row: per-core permuted layout, own shard first
    g1 = np.zeros((P, T * BLK), np.int32)
    s_own_s = s_own[order]
    s_loc_s = s_loc[order]
    for k in range(P):
        sel = own_s == k
        so = s_own_s[sel]
        # position of shard `so` in core k's permuted order [k, 0,1,..(!k)..,7]
        rank = np.where(so == k, 0, 1 + so - (so > k))
        g1.reshape(-1)[flat[sel]] = (rank * PADS + s_loc_s[sel]).astype(np.int32)

    x_pad = np.zeros((P, PADS, IN_DIM), np.float32)
    deg_pad = np.ones((P, PADS), np.float32)
    x_pad[newid // PADS, newid % PADS] = x
    deg_pad[newid // PADS, newid % PADS] = deg

    per_core = []
    for k in range(P):
        perm = [k] + [c for c in range(P) if c != k]
        xp = x_pad[perm].reshape(TBL, IN_DIM)
        degp = deg_pad[perm].reshape(P * NBLK, BLK).T   # [128, 392]
        per_core.append(dict(
            xpt=np.ascontiguousarray(xp.T),
            degp=np.ascontiguousarray(degp),
            g1=np.ascontiguousarray(g1[k].reshape(T, BLK).T.astype(np.int32)),
            g23=np.ascontiguousarray(g23[k].reshape(T, BLK).T.astype(np.int32)),
            dstl=np.ascontiguousarray(dv[k].reshape(T, BLK).T),
        ))
    return per_core, tuple(int(c) for c in C_arr), newid


# ------------------------------------------------------------- device build
def _build(C_arr):
    T = int(sum(C_arr))
    c_base = [0]
    for c in C_arr[:-1]:
        c_base.append(c_base[-1] + c)
    NFULL = P * NBLK          # 392 blocks in the full node space

    nc = bacc.Bacc("TRN2", target_bir_lowering=False, debug=False,
                   enable_asserts=False, num_devices=P,
                   dynamic_dma_scratch_size=65536)

    xpt_d = nc.dram_tensor("xpt", [IN_DIM, TBL], F32, kind="ExternalInput").ap()
    degp_d = nc.dram_tensor("degp", [BLK, NFULL], F32, kind="ExternalInput").ap()
    g1_d = nc.dram_tensor("g1", [BLK, T], I32, kind="ExternalInput").ap()
    g23_d = nc.dram_tensor("g23", [BLK, T], I32, kind="ExternalInput").ap()
    dstl_d = nc.dram_tensor("dstl", [BLK, T], F32, kind="ExternalInput").ap()
    w_d = [nc.dram_tensor(f"w{i}", [d, HID if i < 3 else OUT_DIM], F32,
                          kind="ExternalInput").ap()
           for i, d in enumerate([IN_DIM, HID, HID, HID])]
    bt_d = [nc.dram_tensor(f"bt{i}", [BLK, HID if i < 3 else OUT_DIM], F32,
                           kind="ExternalInput").ap()
            for i in range(4)]
    iota_d = nc.dram_tensor("iota", [BLK, BLK], F32, kind="ExternalInput").ap()
    iden_d = nc.dram_tensor("iden", [BLK, BLK], F32, kind="ExternalInput").ap()
    out_d = nc.dram_tensor("probs", [PADS, OUT_DIM], F32, kind="ExternalOutput").ap()

    rg = [list(range(P))]

    with tile.TileContext(nc) as tc:
        with (
            tc.tile_pool(name="const", bufs=1) as cp,
            tc.tile_pool(name="xin", bufs=3) as xp_pool,
            tc.tile_pool(name="ht", bufs=3) as hp,
            tc.tile_pool(name="zt", bufs=3) as zp,
            tc.tile_pool(name="oh", bufs=12) as ohp,
            tc.tile_pool(name="msg", bufs=32) as mp,
            tc.tile_pool(name="fin", bufs=2) as fp,
            tc.tile_pool(name="pstp", bufs=2, space="PSUM") as pstp,
            tc.tile_pool(name="psacc", bufs=4, space="PSUM") as psacc,
            tc.tile_pool(name="dram", bufs=1, space="DRAM") as dp,
        ):
            # ---- constants into SBUF
            w_sb, bt_sb = [], []
            for i in range(4):
                wt = cp.tile(list(w_d[i].shape), F32, tag=f"w{i}", name=f"w{i}")
                nc.sync.dma_start(wt[:], w_d[i])
                w_sb.append(wt)
                bt = cp.tile(list(bt_d[i].shape), F32, tag=f"bt{i}", name=f"bt{i}")
                nc.sync.dma_start(bt[:], bt_d[i])
                bt_sb.append(bt)
            iota_sb = cp.tile([BLK, BLK], F32, tag="iota")
            nc.sync.dma_start(iota_sb[:], iota_d)
            iden_sb = cp.tile([BLK, BLK], F32, tag="iden")
            nc.sync.dma_start(iden_sb[:], iden_d)
            g1_sb = cp.tile([BLK, T], I32, tag="g1")
            nc.sync.dma_start(g1_sb[:], g1_d)
            g23_sb = cp.tile([BLK, T], I32, tag="g23")
            nc.sync.dma_start(g23_sb[:], g23_d)
            dstl_sb = cp.tile([BLK, T], F32, tag="dstl")
            nc.sync.dma_start(dstl_sb[:], dstl_d)

            deg_sb = cp.tile([BLK, NFULL], F32, tag="deg")
            nc.sync.dma_start(deg_sb[:], degp_d)
            dinv_sb = cp.tile([BLK, NFULL], F32, tag="dinv")
            nc.vector.reciprocal(dinv_sb[:], deg_sb[:])
            nc.scalar.activation(dinv_sb[:], dinv_sb[:],
                                 mybir.ActivationFunctionType.Sqrt)

            h_sb = [cp.tile([BLK, NBLK * HID], F32, tag=f"h{i}", name=f"h{i}")
                    for i in range(2)]
            zt_own = cp.tile([BLK, NBLK * HID], F32, tag="zt_own")

            def transform_block(src_ap, d_in, w_t, b, zdst):
                """z~_block = dinv[:,b] * (src_block @ W) -> zdst [128, HID]"""
                tp_ps = pstp.tile([d_in, BLK], F32, tag="tp", name="tp")
                nc.tensor.transpose(tp_ps[:], src_ap, iden_sb[:])
                hT = hp.tile([d_in, BLK], F32, tag="hT", name="hT")
                nc.vector.tensor_copy(hT[:], tp_ps[:])
                z_ps = psacc.tile([BLK, HID], F32, tag="acc", name="z_ps")
                nc.tensor.matmul(z_ps[:], hT[:], w_t[:], start=True, stop=True)
                nc.vector.tensor_scalar(zdst, z_ps[:], dinv_sb[:, b:b + 1],
                                        None, mybir.AluOpType.mult)

            def readout_block(h_ap, b):
                tp_ps = pstp.tile([HID, BLK], F32, tag="tp", name="tp")
                nc.tensor.transpose(tp_ps[:], h_ap, iden_sb[:])
                hT = hp.tile([HID, BLK], F32, tag="hT", name="hT")
                nc.vector.tensor_copy(hT[:], tp_ps[:])
                o_ps = psacc.tile([BLK, OUT_DIM], F32, tag="acc", name="o_ps")
                nc.tensor.matmul(o_ps[:], hT[:], w_sb[3][:],
                                 start=True, stop=True)
                logit = fp.tile([BLK, OUT_DIM], F32, tag="logit", name="logit")
                nc.vector.tensor_tensor(logit[:], o_ps[:], bt_sb[3][:],
                                        mybir.AluOpType.add)
                nmx = fp.tile([BLK, 1], F32, tag="nmx", name="nmx")
                nc.vector.reduce_max(nmx[:], logit[:],
                                     axis=mybir.AxisListType.X, negate=True)
                ex = fp.tile([BLK, OUT_DIM], F32, tag="ex", name="ex")
                ssum = fp.tile([BLK, 1], F32, tag="ssum", name="ssum")
                nc.scalar.activation(ex[:], logit[:],
                                     mybir.ActivationFunctionType.Exp,
                                     bias=nmx[:], accum_out=ssum[:])
                rs = fp.tile([BLK, 1], F32, tag="rs", name="rs")
                nc.vector.reciprocal(rs[:], ssum[:])
                prob = fp.tile([BLK, OUT_DIM], F32, tag="prob", name="prob")
                nc.vector.tensor_scalar(prob[:], ex[:], rs[:], None,
                                        mybir.AluOpType.mult)
                nc.sync.dma_start(out_d[b * BLK:(b + 1) * BLK, :], prob[:])

            def propagate(gidx_sb, table, h_nxt, b_t, readout=False):
                for b in range(NBLK):
                    C_b = C_arr[b]
                    agg_ps = psacc.tile([BLK, HID], F32, tag="acc", name="agg_ps")
                    for c in range(C_b):
                        t = c_base[b] + c
                        msg = mp.tile([BLK, HID], F32, tag="msg", name="msg")
                        nc.gpsimd.indirect_dma_start(
                            out=msg[:], out_offset=None, in_=table[:],
                            in_offset=bass.IndirectOffsetOnAxis(
                                ap=gidx_sb[:, t:t + 1], axis=0))
                        oh = ohp.tile([BLK, BLK], F32, tag="oh", name="oh")
                        nc.vector.tensor_scalar(
                            oh[:], iota_sb[:], dstl_sb[:, t:t + 1], None,
                            mybir.AluOpType.is_equal)
                        nc.tensor.matmul(agg_ps[:], oh[:], msg[:],
                                         start=(c == 0), stop=(c == C_b - 1))
                    sl = slice(b * HID, (b + 1) * HID)
                    tot = zp.tile([BLK, HID], F32, tag="tot", name="tot")
                    nc.vector.tensor_tensor(tot[:], agg_ps[:], zt_own[:, sl],
                                            mybir.AluOpType.add)
                    nc.vector.scalar_tensor_tensor(
                        h_nxt[:, sl], tot[:], dinv_sb[:, b:b + 1], b_t[:],
                        mybir.AluOpType.mult, mybir.AluOpType.add)
                    nc.scalar.activation(h_nxt[:, sl], h_nxt[:, sl],
                                         mybir.ActivationFunctionType.Relu)
                    if readout:
                        readout_block(h_nxt[:, sl], b)

            # ---------------- layer 1: full local table (x replicated)
            # batched 8-block staging keeps the sync engine off the critical
            # path (one 512KB load + one 256KB store per 8 blocks)
            table1 = dp.tile([TBL, HID], F32, tag="tbl0")
            GB = 8
            for g in range(NFULL // GB):
                # x arrives pre-transposed: columns are nodes, so each block
                # slice is directly the matmul's stationary operand
                xg = xp_pool.tile([IN_DIM, GB * BLK], F32, tag="xb", name="xb")
                nc.sync.dma_start(xg[:], xpt_d[:, g * GB * BLK:(g + 1) * GB * BLK])
                zg = zp.tile([BLK, GB * HID], F32, tag="zd", name="zd")
                for j in range(GB):
                    b = g * GB + j
                    z_ps = psacc.tile([BLK, HID], F32, tag="acc", name="z_ps")
                    nc.tensor.matmul(z_ps[:], xg[:, j * BLK:(j + 1) * BLK],
                                     w_sb[0][:], start=True, stop=True)
                    nc.vector.tensor_scalar(zg[:, j * HID:(j + 1) * HID],
                                            z_ps[:], dinv_sb[:, b:b + 1],
                                            None, mybir.AluOpType.mult)
                    if b < NBLK:
                        nc.vector.tensor_copy(
                            zt_own[:, b * HID:(b + 1) * HID],
                            zg[:, j * HID:(j + 1) * HID])
                nc.sync.dma_start(
                    table1[g * GB * BLK:(g + 1) * GB * BLK, :].rearrange(
                        "(j p) f -> p j f", p=BLK),
                    zg[:].rearrange("p (j f) -> p j f", f=HID))
            propagate(g1_sb, table1, h_sb[0], bt_sb[0])

            # ---------------- layers 2, 3: shard transform + AllGather
            for li in (1, 2):
                h_cur = h_sb[(li + 1) % 2]
                h_nxt = h_sb[li % 2]
                for b in range(NBLK):
                    transform_block(h_cur[:, b * HID:(b + 1) * HID], HID,
                                    w_sb[li], b,
                                    zt_own[:, b * HID:(b + 1) * HID])
                ag_in = dp.tile([PADS, HID], F32, tag=f"agin{li}",
                                name=f"agin{li}")
                nc.sync.dma_start(
                    ag_in[:].rearrange("(b p) f -> p b f", p=BLK),
                    zt_own[:].rearrange("p (b f) -> p b f", f=HID))
                table = dp.tile([TBL, HID], F32, tag=f"tbl{li}",
                                name=f"table{li}", addr_space="Shared")
                nc.gpsimd.collective_compute(
                    "AllGather", mybir.AluOpType.bypass, replica_groups=rg,
                    ins=[ag_in.opt()], outs=[table.opt()])
                propagate(g23_sb, table, h_nxt, bt_sb[li], readout=(li == 2))

    nc.compile()
    return nc


# ------------------------------------------------------------- entry point
_CACHE = {}


def _get_program(C_arr):
    if C_arr not in _CACHE:
        _CACHE[C_arr] = _build(C_arr)
    return _CACHE[C_arr]


def kernel(x, edge_index, W1, b1, W2, b2, W3, b3, Wr, br, trace=False):
    per_core, C_arr, newid = _preprocess(x, edge_index)
    nc = _get_program(C_arr)

    ws = [np.asarray(w, np.float32) for w in (W1, W2, W3, Wr)]
    bts = [np.tile(np.asarray(b, np.float32).reshape(1, -1), (BLK, 1))
           for b in (b1, b2, b3, br)]
    iota = np.tile(np.arange(BLK, dtype=np.float32), (BLK, 1))
    iden = np.eye(BLK, dtype=np.float32)

    in_maps = []
    for k in range(P):
        m = dict(per_core[k])
        for i in range(4):
            m[f"w{i}"] = ws[i]
            m[f"bt{i}"] = bts[i]
        m["iota"] = iota
        m["iden"] = iden
        in_maps.append(m)

    res = run_bass_kernel_spmd(nc, in_maps, core_ids=list(range(P)),
                               trace=trace)
    allp = np.concatenate([res.results[k]["probs"] for k in range(P)], axis=0)
    out = allp[newid]
    kernel.last_results = res
    return out



# revision 9
# speedup vs baseline: 1.1062x; 1.1062x over previous
"""GCN (3-layer + readout) on 8 Trainium2 NeuronCores.

Strategy (dst-node sharding, 1D graph parallel):
  - Nodes are sharded across 8 cores (6250/core, padded to 6272 = 49 blocks
    of 128).  Each core aggregates messages for the edges whose dst lands in
    its shard.  Self-loops are materialized as explicit edges (i, i): their
    gathered message dinv_i*z_i times the final dst-side dinv_i scale equals
    the reference's implicit dinv^2 self term exactly.
  - Everything on the message path is fp16 (the rel-err budget is 2e-2).
  - Per layer: transform z = h @ W on the PE, scale rows by dinv = deg^-1/2
    (fused into the Scalar-engine PSUM->SBUF copy) so table rows are
    dinv[src]*z[src].  Layer 1's table is computed fully locally by every
    core (x is replicated); layers 2/3 transform the own shard and AllGather
    the shard tables.  Table rows are padded to 128 fp16 (256B) to satisfy
    the gather ucode's element-size constraint; the pad columns are never
    read.
  - Edge gathers: ONE dma_gather (InstDMAGatherAnt) per G=16 chunks of 128
    edges — the ~1us fixed SWDGE cost is amortized 16x vs one indirect DMA
    per chunk.  Indices are int16, so the table is processed in two halves
    (row ids < 25088 each); every dst block accumulates in two passes with a
    fp16 partial buffer in between.  Edges are sorted by src row inside each
    (core, half, dst-block) bucket for HBM locality.
  - Scatter-add on the TensorEngine with messages stationary:
    psum[64 feat, 128 dst] += msg[128e, 64f]^T @ onehot[128e, 128d], so the
    block aggregate lands feature-major and feeds the next layer's
    transform (lhsT = hT block) with no transposes anywhere.
  - One-hot matrices are static per graph: precomputed on the host, stored
    e-major in DRAM, and streamed per group with a single contiguous HWDGE
    DMA (4KB per partition) instead of being built on the Vector engine.
  - dst-side dinv scale happens in feature-major space via a precomputed
    broadcast tile dinvb[64, PADS] (rank-1 PE matmuls of ones x dinv row).
  - Host-side preprocessing is strictly index/metadata work (edge bucketing,
    padding, degree counting); all float math runs on device.
"""

import numpy as np

from concourse import bacc, bass, mybir, tile
from concourse.bass_utils import run_bass_kernel_spmd

# ---------------------------------------------------------------- constants
P = 8                      # cores
N = 50000                  # nodes
IN_DIM = 128
HID = 64
OUT_DIM = 10
BLK = 128
G = 16                     # chunks per dma_gather / onehot-stream group

F32 = mybir.dt.float32
F16 = mybir.dt.float16
I16 = mybir.dt.int16

SHARD = N // P
NBLK = (SHARD + BLK - 1) // BLK      # 49
PADS = NBLK * BLK                    # 6272
TBL = P * PADS                       # 50176
NFULL = P * NBLK                     # 392
HALF = TBL // 2                      # 25088 (< int16 max)


# ------------------------------------------------------------- host prep
def _preprocess(x, edge_index):
    """Bucket edges (incl. one self-edge per node) into per-(core, half,
    dst-block) 128-edge chunks.

    Nodes are bin-packed into the P*NBLK (core, block) bins by in-degree
    (capacity-constrained LPT) so every bin carries ~the same edge count —
    this minimizes the uniform per-block chunk counts, which set the Q7
    gather-instruction floor.
    """
    import heapq

    x = np.asarray(x, np.float32)
    ei = np.asarray(edge_index, np.int64)
    src, dst = ei[0], ei[1]

    degE = np.bincount(dst, minlength=N).astype(np.int64)
    deg = (degE + 1).astype(np.float32)

    NBINS = P * NBLK
    order_n = np.argsort(-degE, kind="stable")
    heap = [(0, b) for b in range(NBINS)]
    heapq.heapify(heap)
    fill = np.zeros(NBINS, np.int64)
    node_bin = np.empty(N, np.int64)
    node_slot = np.empty(N, np.int64)
    for n in order_n:
        while True:
            s, b = heapq.heappop(heap)
            if fill[b] < BLK:
                break
        node_bin[n] = b
        node_slot[n] = fill[b]
        fill[b] += 1
        heapq.heappush(heap, (s + int(degE[n]) + 1, b))

    newid = node_bin * BLK + node_slot          # padded global row of each node

    # edge stream = input edges + one self edge per node
    all_src = np.concatenate([src, np.arange(N, dtype=np.int64)])
    all_dst = np.concatenate([dst, np.arange(N, dtype=np.int64)])

    rows = newid[all_src]
    half = rows // HALF
    lrow = (rows - half * HALF).astype(np.int16)
    owner = node_bin[all_dst] // NBLK
    blk = node_bin[all_dst] % NBLK
    dstl = node_slot[all_dst].astype(np.int64)

    # bucket + in-bucket src sort (HBM locality for the gather descriptors)
    key = (owner * 2 + half) * NBLK + blk
    order = np.lexsort((rows, key))
    key_s = key[order]
    counts = np.bincount(key_s, minlength=P * 2 * NBLK)
    starts = np.concatenate([[0], np.cumsum(counts)[:-1]])
    pos = np.arange(key_s.size) - starts[key_s]

    # per-(half, block) chunk count: max over cores (program is core-uniform)
    C2 = np.maximum(np.ceil(
        counts.reshape(P, 2 * NBLK).max(axis=0) / BLK).astype(np.int64), 1)
    base2 = np.concatenate([[0], np.cumsum(C2)[:-1]])   # chunk id of bucket
    T = int(C2.sum())

    own_s = key_s // (2 * NBLK)
    hb_s = key_s % (2 * NBLK)
    slot = base2[hb_s] * BLK + pos            # (chunk, lane) within the stream
    flat = own_s * (T * BLK) + slot

    gidx = np.zeros((P, T * BLK), np.int16)
    gidx.reshape(-1)[flat] = lrow[order]
    dv = np.full((P, T * BLK), -1, np.int64)
    dv.reshape(-1)[flat] = dstl[order]

    # group layout: G-chunk runs inside each half-section
    T0 = int(C2[:NBLK].sum())
    groups = []                               # (lo, hi, sec)
    for sec, (s0, s1) in enumerate(((0, T0), (T0, T))):
        lo = s0
        while lo < s1:
            hi = min(lo + G, s1)
            groups.append((lo, hi, sec))
            lo = hi

    # wrapped int16 index layout for dma_gather: per group, index i (chunk-
    # major within the group) lives at [i % 16, lo*8 + i // 16], replicated
    # across the 8 Q7 cores' partition sets.
    idxw = np.zeros((P, BLK, T * 8), np.int16)
    oh = np.zeros((P, BLK, T * BLK), np.float16)
    lanes = np.arange(T * BLK)
    for k in range(P):
        gk = gidx[k].reshape(T, BLK)
        for (lo, hi, _s) in groups:
            w = gk[lo:hi].reshape(-1).reshape(-1, 16).T     # [16, (hi-lo)*8]
            idxw[k][:, lo * 8:hi * 8] = np.tile(w, (8, 1))
        dvk = dv[k]
        sel = dvk >= 0
        oh[k][lanes[sel] % BLK, (lanes[sel] // BLK) * BLK + dvk[sel]] = 1.0

    x_pad = np.zeros((TBL, IN_DIM), np.float32)
    deg_pad = np.ones((P, PADS), np.float32)
    x_pad[newid] = x
    deg_pad.reshape(-1)[newid] = deg
    xpt = np.ascontiguousarray(x_pad.T.astype(np.float16))          # [128, TBL]
    degp = np.ascontiguousarray(
        deg_pad.reshape(NFULL, BLK).T)                              # [128, 392]

    per_core = []
    for k in range(P):
        per_core.append(dict(
            xpt=xpt,
            degp=degp,
            dego=np.ascontiguousarray(degp[:, k * NBLK:(k + 1) * NBLK]),
            degbt=np.ascontiguousarray(deg_pad[k].reshape(1, PADS)),
            idxw=np.ascontiguousarray(idxw[k]),
            ohd=np.ascontiguousarray(oh[k]),
        ))
    meta = (tuple(int(c) for c in C2), tuple(groups))
    return per_core, meta, newid


# ------------------------------------------------------------- device build
def _build(meta):
    C2, groups = meta
    T = int(sum(C2))
    base2 = [0]
    for c in C2[:-1]:
        base2.append(base2[-1] + c)
    # chunk id -> (group index, offset within group)
    gmap = {}
    for gi, (lo, hi, _s) in enumerate(groups):
        for t in range(lo, hi):
            gmap[t] = (gi, t - lo)

    nc = bacc.Bacc("TRN2", target_bir_lowering=False, debug=False,
                   enable_asserts=False, num_devices=P,
                   dynamic_dma_scratch_size=65536)

    xpt_d = nc.dram_tensor("xpt", [IN_DIM, TBL], F16, kind="ExternalInput").ap()
    degp_d = nc.dram_tensor("degp", [BLK, NFULL], F32, kind="ExternalInput").ap()
    dego_d = nc.dram_tensor("dego", [BLK, NBLK], F32, kind="ExternalInput").ap()
    degbt_d = nc.dram_tensor("degbt", [1, PADS], F32, kind="ExternalInput").ap()
    idxw_d = nc.dram_tensor("idxw", [BLK, T * 8], I16, kind="ExternalInput").ap()
    ohd_d = nc.dram_tensor("ohd", [BLK, T * BLK], F16, kind="ExternalInput").ap()
    w_d = [nc.dram_tensor(f"w{i}", [d, HID if i < 3 else OUT_DIM], F16,
                          kind="ExternalInput").ap()
           for i, d in enumerate([IN_DIM, HID, HID, HID])]
    bc_d = [nc.dram_tensor(f"bc{i}", [HID, 1], F32, kind="ExternalInput").ap()
            for i in range(3)]
    btr_d = nc.dram_tensor("btr", [BLK, OUT_DIM], F32, kind="ExternalInput").ap()
    out_d = nc.dram_tensor("probs", [PADS, OUT_DIM], F32, kind="ExternalOutput").ap()

    rg = [list(range(P))]

    with tile.TileContext(nc) as tc:
        with (
            tc.tile_pool(name="const", bufs=1) as cp,
            tc.tile_pool(name="xin", bufs=3) as xp_pool,
            tc.tile_pool(name="zt", bufs=3) as zp,
            tc.tile_pool(name="oh", bufs=4) as ohp,
            tc.tile_pool(name="msg", bufs=4) as mp,
            tc.tile_pool(name="cmb", bufs=4) as cb,
            tc.tile_pool(name="fin", bufs=2) as fp,
            tc.tile_pool(name="part", bufs=2) as pp,
            tc.tile_pool(name="psz", bufs=3, space="PSUM") as psz,
            tc.tile_pool(name="psacc", bufs=3, space="PSUM") as psacc,
            tc.tile_pool(name="pso", bufs=1, space="PSUM") as pso,
            tc.tile_pool(name="dram", bufs=1, space="DRAM") as dp,
        ):
            # ---- constants into SBUF
            w_sb, bc_sb = [], []
            for i in range(4):
                wt = cp.tile(list(w_d[i].shape), F16, tag=f"w{i}", name=f"w{i}")
                nc.sync.dma_start(wt[:], w_d[i])
                w_sb.append(wt)
            for i in range(3):
                bt = cp.tile([HID, 1], F32, tag=f"bc{i}", name=f"bc{i}")
                nc.sync.dma_start(bt[:], bc_d[i])
                bc_sb.append(bt)
            btr_sb = cp.tile([BLK, OUT_DIM], F32, tag="btr")
            nc.sync.dma_start(btr_sb[:], btr_d)
            idxw_sb = cp.tile([BLK, T * 8], I16, tag="idxw")
            nc.sync.dma_start(idxw_sb[:], idxw_d)

            # dinv = deg^-1/2 in the three layouts we need
            dinv_sb = cp.tile([BLK, NFULL], F32, tag="dinv")
            nc.sync.dma_start(dinv_sb[:], degp_d)
            nc.vector.reciprocal(dinv_sb[:], dinv_sb[:])
            nc.scalar.activation(dinv_sb[:], dinv_sb[:],
                                 mybir.ActivationFunctionType.Sqrt)
            dinvo_sb = cp.tile([BLK, NBLK], F32, tag="dinvo")
            nc.sync.dma_start(dinvo_sb[:], dego_d)
            nc.vector.reciprocal(dinvo_sb[:], dinvo_sb[:])
            nc.scalar.activation(dinvo_sb[:], dinvo_sb[:],
                                 mybir.ActivationFunctionType.Sqrt)
            dinvbt_sb = cp.tile([1, PADS], F32, tag="dinvbt")
            nc.sync.dma_start(dinvbt_sb[:], degbt_d)
            nc.vector.reciprocal(dinvbt_sb[:], dinvbt_sb[:])
            nc.scalar.activation(dinvbt_sb[:], dinvbt_sb[:],
                                 mybir.ActivationFunctionType.Sqrt)

            # dinvb[64, PADS] (f16): dst-side dinv broadcast across the
            # feature partitions, built with rank-1 matmuls ones^T x dinv_row
            ones_sb = cp.tile([1, HID], F32, tag="ones")
            nc.vector.memset(ones_sb[:], 1.0)
            dinvb_sb = cp.tile([HID, PADS], F16, tag="dinvb")
            for i in range(PADS // 448):
                off = i * 448
                ps = pso.tile([HID, 448], F32, tag="bc", name="bc_ps")
                nc.tensor.matmul(ps[:], ones_sb[:], dinvbt_sb[:, off:off + 448],
                                 start=True, stop=True)
                nc.vector.tensor_copy(dinvb_sb[:, off:off + 448], ps[:])

            hT = [cp.tile([HID, PADS], F16, tag=f"h{i}", name=f"h{i}")
                  for i in range(2)]

            # ---------------- layer 1 table: full local transform (x replicated)
            table1 = dp.tile([TBL, BLK], F16, tag="tbl0")
            GB = 8
            for g in range(NFULL // GB):
                xg = xp_pool.tile([IN_DIM, GB * BLK], F16, tag="xb", name="xb")
                nc.sync.dma_start(xg[:], xpt_d[:, g * GB * BLK:(g + 1) * GB * BLK])
                zg = zp.tile([BLK, GB * BLK], F16, tag="zd1", name="zd1")
                for j in range(GB):
                    b = g * GB + j
                    z_ps = psz.tile([BLK, HID], F32, tag="z", name="z_ps")
                    nc.tensor.matmul(z_ps[:], xg[:, j * BLK:(j + 1) * BLK],
                                     w_sb[0][:], start=True, stop=True)
                    nc.scalar.activation(zg[:, j * BLK:j * BLK + HID], z_ps[:],
                                         mybir.ActivationFunctionType.Copy,
                                         scale=dinv_sb[:, b:b + 1])
                nc.sync.dma_start(
                    table1[g * GB * BLK:(g + 1) * GB * BLK, :].rearrange(
                        "(j p) f -> p j f", p=BLK),
                    zg[:].rearrange("p (j f) -> p j f", f=BLK))

            def readout_block(h_ap, b):
                o_ps = pso.tile([BLK, OUT_DIM], F32, tag="o", name="o_ps")
                nc.tensor.matmul(o_ps[:], h_ap, w_sb[3][:], start=True, stop=True)
                logit = fp.tile([BLK, OUT_DIM], F32, tag="logit", name="logit")
                nc.vector.tensor_tensor(logit[:], o_ps[:], btr_sb[:],
                                        mybir.AluOpType.add)
                nmx = fp.tile([BLK, 1], F32, tag="nmx", name="nmx")
                nc.vector.reduce_max(nmx[:], logit[:],
                                     axis=mybir.AxisListType.X, negate=True)
                ex = fp.tile([BLK, OUT_DIM], F32, tag="ex", name="ex")
                ssum = fp.tile([BLK, 1], F32, tag="ssum", name="ssum")
                nc.scalar.activation(ex[:], logit[:],
                                     mybir.ActivationFunctionType.Exp,
                                     bias=nmx[:], accum_out=ssum[:])
                rs = fp.tile([BLK, 1], F32, tag="rs", name="rs")
                nc.vector.reciprocal(rs[:], ssum[:])
                prob = fp.tile([BLK, OUT_DIM], F32, tag="prob", name="prob")
                nc.vector.tensor_scalar(prob[:], ex[:], rs[:], None,
                                        mybir.AluOpType.mult)
                nc.sync.dma_start(out_d[b * BLK:(b + 1) * BLK, :], prob[:])

            def propagate(table, h_out, bc_t, readout=False):
                loaded = {}

                def group_tiles(gi):
                    if gi in loaded:
                        return loaded[gi]
                    lo, hi, sec = groups[gi]
                    n = hi - lo
                    m = mp.tile([BLK, G * BLK], F16, tag="msg", name="msg")
                    nc.gpsimd.dma_gather(
                        m[:, :n * BLK].rearrange("p (g f) -> p g f", f=BLK),
                        table[sec * HALF:(sec + 1) * HALF, :],
                        idxw_sb[:, lo * 8:hi * 8],
                        num_idxs=n * BLK, num_idxs_reg=n * BLK,
                        elem_size=BLK, single_packet=False)
                    oh = ohp.tile([BLK, G * BLK], F16, tag="oh", name="oh")
                    nc.sync.dma_start(oh[:, :n * BLK],
                                      ohd_d[:, lo * BLK:hi * BLK])
                    loaded[gi] = (m, oh)
                    return loaded[gi]

                partial = pp.tile([HID, PADS], F16, tag="pt", name="partial")
                for sec in (0, 1):
                    for b in range(NBLK):
                        C_b = C2[sec * NBLK + b]
                        cb0 = base2[sec * NBLK + b]
                        agg = psacc.tile([HID, BLK], F32, tag="acc", name="agg")
                        for c in range(C_b):
                            t = cb0 + c
                            gi, r = gmap[t]
                            m, oh = group_tiles(gi)
                            nc.tensor.matmul(agg[:],
                                             m[:, r * BLK:r * BLK + HID],
                                             oh[:, r * BLK:(r + 1) * BLK],
                                             start=(c == 0), stop=(c == C_b - 1))
                        sl = slice(b * BLK, (b + 1) * BLK)
                        if sec == 0:
                            nc.vector.tensor_copy(partial[:, sl], agg[:])
                        else:
                            tot = cb.tile([HID, BLK], F32, tag="tot", name="tot")
                            nc.vector.tensor_tensor(tot[:], agg[:],
                                                    partial[:, sl],
                                                    mybir.AluOpType.add)
                            tmp = cb.tile([HID, BLK], F16, tag="tmp", name="tmp")
                            nc.vector.tensor_tensor(tmp[:], tot[:],
                                                    dinvb_sb[:, sl],
                                                    mybir.AluOpType.mult)
                            nc.scalar.activation(h_out[:, sl], tmp[:],
                                                 mybir.ActivationFunctionType.Relu,
                                                 bias=bc_t[:])
                            if readout:
                                readout_block(h_out[:, sl], b)

            propagate(table1, hT[0], bc_sb[0])

            # ---------------- layers 2, 3: shard transform + AllGather
            TGB = 7
            for li in (1, 2):
                h_cur = hT[(li + 1) % 2]
                h_nxt = hT[li % 2]
                ag_in = dp.tile([PADS, BLK], F16, tag=f"agin{li}",
                                name=f"agin{li}")
                for g in range(NBLK // TGB):
                    zg = zp.tile([BLK, TGB * BLK], F16, tag="zd2", name="zd2")
                    for j in range(TGB):
                        b = g * TGB + j
                        z_ps = psz.tile([BLK, HID], F32, tag="z", name="z_ps")
                        nc.tensor.matmul(z_ps[:],
                                         h_cur[:, b * BLK:(b + 1) * BLK],
                                         w_sb[li][:], start=True, stop=True)
                        nc.scalar.activation(zg[:, j * BLK:j * BLK + HID],
                                             z_ps[:],
                                             mybir.ActivationFunctionType.Copy,
                                             scale=dinvo_sb[:, b:b + 1])
                    nc.sync.dma_start(
                        ag_in[g * TGB * BLK:(g + 1) * TGB * BLK, :].rearrange(
                            "(j p) f -> p j f", p=BLK),
                        zg[:].rearrange("p (j f) -> p j f", f=BLK))
                table = dp.tile([TBL, BLK], F16, tag=f"tbl{li}",
                                name=f"table{li}", addr_space="Shared")
                nc.gpsimd.collective_compute(
                    "AllGather", mybir.AluOpType.bypass, replica_groups=rg,
                    ins=[ag_in.opt()], outs=[table.opt()])
                propagate(table, h_nxt, bc_sb[li], readout=(li == 2))

    nc.compile()
    return nc


# ------------------------------------------------------------- entry point
_CACHE = {}


def _get_program(meta):
    if meta not in _CACHE:
        _CACHE[meta] = _build(meta)
    return _CACHE[meta]


def kernel(x, edge_index, W1, b1, W2, b2, W3, b3, Wr, br, trace=False):
    per_core, meta, newid = _preprocess(x, edge_index)
    nc = _get_program(meta)

    ws = [np.asarray(w, np.float16) for w in (W1, W2, W3, Wr)]
    bcs = [np.asarray(b, np.float32).reshape(HID, 1) for b in (b1, b2, b3)]
    btr = np.tile(np.asarray(br, np.float32).reshape(1, -1), (BLK, 1))

    in_maps = []
    for k in range(P):
        m = dict(per_core[k])
        for i in range(4):
            m[f"w{i}"] = ws[i]
        for i in range(3):
            m[f"bc{i}"] = bcs[i]
        m["btr"] = btr
        in_maps.append(m)

    res = run_bass_kernel_spmd(nc, in_maps, core_ids=list(range(P)),
                               trace=trace)
    allp = np.concatenate([res.results[k]["probs"] for k in range(P)], axis=0)
    out = allp[newid]
    kernel.last_results = res
    return out
